# revision 1
# baseline (speedup 1.0000x reference)
"""Trainium2 Bass kernel for nn_CoucheinitialeGNN (GNN edge-MLP + segment-sum normalize).

Math (reference):
    bucket = clip(int(dist), 0, 9); one_hot [E,10]
    h      = relu(dist @ W1 + b1)          [E,128]
    mlp    = relu(h @ W2 + b2)             [E,54]
    w      = concat([one_hot, mlp])        [E,64]
    d      = segment_sum(w, src, N)        [N,64]
    out    = w / d[src]   (0/0 := 0)       [E,64]

Strategy: sort edges by src on host; shard nodes across 8 cores; within a
core, snake-pack nodes into NB bins of <=128 nodes / <=TPB*128 edge slots.
Per bin, a 0/1 selection matrix S (and its transpose) is built on-device from
exact bf16 outer-difference matmuls + is_equal. The segment-sum is a
PSUM-accumulated f32 matmul (S contract w); the per-edge reciprocal is
expanded back with two bf16 matmuls (S^T contract [r_hi, r_lo]). Layer-1 of
the MLP is one bf16 matmul made ~f32-exact by a 3-way hi/mid/lo split of dist
and the weights; always-linear/dead relu features are folded on the host so
layer-2 (exact f32 matmul) fits in 128 contraction rows with dist/ones rows
appended.
"""

import numpy as np
import ml_dtypes

import concourse.bass as bass
import concourse.bacc as bacc
import concourse.tile as tile
import concourse.mybir as mybir
from concourse.bass_utils import run_bass_kernel_spmd

F32 = mybir.dt.float32
BF16 = mybir.dt.bfloat16

N_NODES = 100000
N_EDGES = 1600000
N_CORES = 8
THRESHOLD = 10.0

NODES_PER_CORE = N_NODES // N_CORES          # 12500
NB = 99                                      # bins per core (99*128 >= 12500)
TPB = 16                                     # tiles (of 128 edge slots) per bin
EPS = 1e-38


def _epb():
    return TPB * 128


def _ep():
    return NB * _epb()


# ---------------------------------------------------------------------------
# host-side weight folding
# ---------------------------------------------------------------------------

def fold_weights(W1, b1, W2, b2):
    """Split relu features into kinky (computed on device) and linear/dead
    (folded into two extra contraction rows: dist-coef and const)."""
    W1 = np.asarray(W1, np.float32).reshape(-1)       # [128]
    b1 = np.asarray(b1, np.float32).reshape(-1)       # [128]
    W2 = np.asarray(W2, np.float32)                   # [128, 54]
    b2 = np.asarray(b2, np.float32).reshape(-1)       # [54]
    H, O = W2.shape

    lo = b1                                            # value at d -> 0+
    hi = THRESHOLD * W1 + b1                           # value at d = 10
    with np.errstate(divide="ignore", invalid="ignore"):
        t = np.where(W1 != 0, -b1 / W1, np.inf)
    kinky = (t > -0.5) & (t < THRESHOLD + 0.5) & (W1 != 0)
    dead = ~kinky & (np.maximum(lo, hi) <= 0)
    linear = ~kinky & ~dead                            # relu == identity on (0,10]

    # promote as many linear features as fit into the device-computed set:
    # relu is exact-identity for them, and keeping them on-device removes the
    # fold-skips-f32-rounding error that perturbs knife-edge relu channels.
    room = 128 - 2 - int(kinky.sum())
    lin_idx = np.nonzero(linear)[0]
    if len(lin_idx) > room:
        impact = np.maximum(np.abs(lo), np.abs(hi))[lin_idx] *             np.abs(W2[lin_idx]).max(1)
        lin_idx = lin_idx[np.argsort(-impact)[:room]]
    promote = np.zeros_like(linear)
    promote[lin_idx] = True
    kinky = kinky | promote
    linear = linear & ~promote

    KH = int(kinky.sum())
    assert KH + 2 <= 128, f"kinky feature count {KH} too large"

    A = (W2[linear].astype(np.float64) * W1[linear, None].astype(np.float64)).sum(0)
    C = (W2[linear].astype(np.float64) * b1[linear, None].astype(np.float64)).sum(0) \
        + b2.astype(np.float64)

    # layer-1 lhsT [9, KH+2] bf16, paired with rhs rows
    # [dh, dh, dh, dm, dm, dl, 1, 1, 1]:
    #   col j<KH: [wh, wm, wl, wh, wm, wh, b1h, b1m, b1l]
    #   col KH:   dist-copy [1,0,0,1,0,1,0,0,0] -> dh+dm+dl = dist
    #   col KH+1: ones-copy [0,...,0,1,0,0]     -> 1
    def split3(v):
        hi_ = v.astype(ml_dtypes.bfloat16)
        r = v - hi_.astype(np.float32)
        mid = r.astype(ml_dtypes.bfloat16)
        lo_ = (r - mid.astype(np.float32)).astype(ml_dtypes.bfloat16)
        return hi_, mid, lo_

    W1k = W1[kinky]
    b1k = b1[kinky]
    wh, wm, wl = split3(W1k)
    bh, bm, bl = split3(b1k)
    F = KH + 2
    l1 = np.zeros((9, F), ml_dtypes.bfloat16)
    for i, row in enumerate([wh, wm, wl, wh, wm, wh, bh, bm, bl]):
        l1[i, :KH] = row
    l1[0, KH] = 1.0
    l1[3, KH] = 1.0
    l1[5, KH] = 1.0
    l1[6, KH + 1] = 1.0

    w2aug = np.zeros((F, O), np.float32)
    w2aug[:KH] = W2[kinky]
    w2aug[KH] = A.astype(np.float32)
    w2aug[KH + 1] = C.astype(np.float32)
    return l1, w2aug, KH


# ---------------------------------------------------------------------------
# host-side edge partitioning
# ---------------------------------------------------------------------------

def plan_bins(src):
    """Snake-pack nodes into bins per core. Returns per-core
    (eids_sorted, edge_bin, edge_rank, ebins) and the required TPB."""
    order = np.argsort(src, kind="stable")
    ssrc = src[order]
    core_bounds = np.searchsorted(ssrc, np.arange(N_CORES + 1) * NODES_PER_CORE)
    plans = []
    max_ebin = 0
    for k in range(N_CORES):
        lo, hi = core_bounds[k], core_bounds[k + 1]
        eids = order[lo:hi]
        lsrc = (ssrc[lo:hi] - k * NODES_PER_CORE).astype(np.int64)
        deg = np.bincount(lsrc, minlength=NODES_PER_CORE)

        nodes = np.argsort(-deg, kind="stable")
        bin_of = np.empty(NODES_PER_CORE, np.int32)
        rank_of = np.empty(NODES_PER_CORE, np.int32)
        counts = np.zeros(NB, np.int32)
        fwd = True
        i = 0
        while i < NODES_PER_CORE:
            rng = range(NB) if fwd else range(NB - 1, -1, -1)
            for b in rng:
                if i >= NODES_PER_CORE:
                    break
                n = nodes[i]
                bin_of[n] = b
                rank_of[n] = counts[b]
                counts[b] += 1
                i += 1
            fwd = not fwd
        assert counts.max() <= 128

        edge_bin = bin_of[lsrc]
        edge_rank = rank_of[lsrc]
        ebins = np.bincount(edge_bin, minlength=NB)
        max_ebin = max(max_ebin, int(ebins.max()))
        plans.append((eids, edge_bin, edge_rank, ebins))
    tpb = max(1, -(-max_ebin // 128))
    return plans, tpb


def prepare_inputs(plans, dist):
    EPB = _epb()
    EP = _ep()
    in_maps = []
    gids_all = []
    for eids, edge_bin, edge_rank, ebins in plans:
        eorder = np.argsort(edge_bin, kind="stable")
        slots_rank = np.zeros(EP, np.int32)
        slot_valid = np.zeros(EP, bool)
        gids = np.full(EP, -1, np.int64)
        dist_s = np.full(EP, 1.0, np.float32)
        bstarts = np.concatenate([[0], np.cumsum(ebins)])
        for b in range(NB):
            seg = eorder[bstarts[b]:bstarts[b + 1]]
            s0 = b * EPB
            n = len(seg)
            slots_rank[s0:s0 + n] = edge_rank[seg]
            slot_valid[s0:s0 + n] = True
            gids[s0:s0 + n] = eids[seg]
            dist_s[s0:s0 + n] = dist[eids[seg]]

        # rhs9 [9, EP] bf16: dh, dh, dh, dm, dm, dl, 1, 1, 1
        dh = dist_s.astype(ml_dtypes.bfloat16)
        r1 = dist_s - dh.astype(np.float32)
        dm = r1.astype(ml_dtypes.bfloat16)
        dl = (r1 - dm.astype(np.float32)).astype(ml_dtypes.bfloat16)
        rhs9 = np.empty((9, EP), ml_dtypes.bfloat16)
        rhs9[0] = dh
        rhs9[1] = dh
        rhs9[2] = dh
        rhs9[3] = dm
        rhs9[4] = dm
        rhs9[5] = dl
        rhs9[6] = 1.0
        rhs9[7] = 1.0
        rhs9[8] = 1.0

        # srcz [2, EP] bf16: srcREL (pad -2), ones
        srcz = np.empty((2, EP), ml_dtypes.bfloat16)
        srel = np.where(slot_valid, slots_rank, -2).astype(np.float32)
        srcz[0] = srel.astype(ml_dtypes.bfloat16)
        srcz[1] = 1.0

        # bucketf [128, NB*TPB] f32: column b*TPB+k, row p = bucket(slot TPB*p+k)
        bucket = np.clip(dist_s.astype(np.int32), 0, 9).astype(np.float32)
        bucketf = (
            bucket.reshape(NB, 128, TPB).transpose(1, 0, 2).reshape(128, NB * TPB)
        )

        in_maps.append({"rhs9": rhs9, "srcz": srcz, "bucketf": bucketf})
        gids_all.append(gids)
    return in_maps, gids_all


# ---------------------------------------------------------------------------
# device kernel
# ---------------------------------------------------------------------------

_NC_CACHE = {}


def build_kernel(KH, l1_np, w2aug_np):
    F = KH + 2
    EPB = _epb()
    EP = _ep()
    nc = bacc.Bacc("TRN2", target_bir_lowering=False, debug=False, num_devices=N_CORES)

    rhs9 = nc.dram_tensor("rhs9", [9, EP], BF16, kind="ExternalInput")
    srcz = nc.dram_tensor("srcz", [2, EP], BF16, kind="ExternalInput")
    bucketf = nc.dram_tensor("bucketf", [128, NB * TPB], F32, kind="ExternalInput")
    out = nc.dram_tensor("out", [EP, 64], F32, kind="ExternalOutput")

    l1_t = nc.inline_tensor(np.ascontiguousarray(l1_np), name="l1w")
    w2_t = nc.inline_tensor(np.ascontiguousarray(w2aug_np), name="w2aug")
    zrhs_np = np.stack(
        [np.ones(128), -np.arange(128, dtype=np.float64)]
    ).astype(ml_dtypes.bfloat16)
    zrhs_t = nc.inline_tensor(zrhs_np, name="zrhs")
    iota10_np = np.broadcast_to(np.arange(10, dtype=np.float32), (128, 10)).copy()
    iota10_t = nc.inline_tensor(iota10_np, name="iota10")

    chunks = []
    c0 = 0
    while c0 < EPB:
        cw = min(512, EPB - c0)
        chunks.append((c0, cw))
        c0 += cw

    with tile.TileContext(nc) as tc:
        with (
            tc.tile_pool(name="const", bufs=1) as cpool,
            tc.tile_pool(name="io", bufs=2) as iopool,
            tc.tile_pool(name="work", bufs=2) as wpool,
            tc.tile_pool(name="ps1", bufs=1, space="PSUM") as ps1p,
            tc.tile_pool(name="psz", bufs=1, space="PSUM") as pszp,
            tc.tile_pool(name="pszt", bufs=1, space="PSUM") as psztp,
            tc.tile_pool(name="psm", bufs=1, space="PSUM") as psmp,
            tc.tile_pool(name="psd", bufs=1, space="PSUM") as psdp,
            tc.tile_pool(name="psrt", bufs=1, space="PSUM") as psrtp,
        ):
            l1c = cpool.tile([9, F], BF16)
            w2c = cpool.tile([F, 54], F32)
            zrc = cpool.tile([2, 128], BF16)
            i10 = cpool.tile([128, 10], F32)
            nc.sync.dma_start(l1c[:], l1_t[:, :])
            nc.sync.dma_start(w2c[:], w2_t[:, :])
            nc.sync.dma_start(zrc[:], zrhs_t[:, :])
            nc.sync.dma_start(i10[:], iota10_t[:, :])

            for b in range(NB):
                e0 = b * EPB
                t_rhs9 = iopool.tile([9, EPB], BF16, tag="rhs9")
                t_srcz = iopool.tile([2, EPB], BF16, tag="srcz")
                t_bk = iopool.tile([128, TPB], F32, tag="bk")
                nc.sync.dma_start(t_rhs9[:], rhs9[:, e0:e0 + EPB])
                nc.sync.dma_start(t_srcz[:], srcz[:, e0:e0 + EPB])
                nc.sync.dma_start(t_bk[:], bucketf[:, b * TPB:(b + 1) * TPB])

                # ---- L1: hT [F, EPB] f32 = relu([W1*d + b1 | dist | ones])
                hT = wpool.tile([F, EPB], F32, tag="hT")
                for ci, (c0_, cw) in enumerate(chunks):
                    p1 = ps1p.tile([F, 512], F32, space="PSUM", tag="p1")
                    nc.tensor.matmul(
                        out=p1[:, :cw],
                        lhsT=l1c[:],
                        rhs=t_rhs9[:, c0_:c0_ + cw],
                        start=True,
                        stop=True,
                    )
                    nc.scalar.activation(
                        hT[:, c0_:c0_ + cw], p1[:, :cw],
                        mybir.ActivationFunctionType.Relu,
                    )

                # ---- zT: const-stationary chunked matmuls over the whole bin
                # zT[n, e] = srcREL_e - n for e in slot order; tile k occupies
                # columns k::TPB, so a 512-col chunk covers tiles 4c..4c+3 via
                # its strided view when sliced per tile below.
                w_t = wpool.tile([128, TPB, 64], F32, tag="w")
                wh_t = wpool.tile([128, TPB, 64], BF16, tag="wh")
                wl_t = wpool.tile([128, TPB, 64], BF16, tag="wl")
                S_t = wpool.tile([128, TPB, 128], BF16, tag="S")
                ST_t = wpool.tile([128, TPB, 128], BF16, tag="ST")
                for g0 in range(0, TPB, 4):
                    gn = min(4, TPB - g0)
                    pzt = psztp.tile([128, 4 * 128], F32, space="PSUM", tag="pzt")
                    # zT chunk: moving = srcz columns of tiles g0..g0+gn
                    # (strided per tile to keep tile-major psum layout)
                    for kk in range(gn):
                        k = g0 + kk
                        nc.tensor.matmul(
                            out=pzt[:, kk * 128:(kk + 1) * 128],
                            lhsT=zrc[:], rhs=t_srcz[:, k::TPB],
                            start=True, stop=True,
                        )
                    nc.vector.tensor_scalar(
                        out=ST_t[:, g0:g0 + gn, :], in0=pzt[:, :gn * 128],
                        scalar1=0.0, scalar2=None, op0=mybir.AluOpType.is_equal,
                    )
                for g0 in range(0, TPB, 4):
                    gn = min(4, TPB - g0)
                    pz = pszp.tile([128, 4 * 128], F32, space="PSUM", tag="pz")
                    pm = psmp.tile([128, 4 * 54], F32, space="PSUM", tag="pm")
                    for kk in range(gn):
                        k = g0 + kk
                        sl = t_srcz[:, k::TPB]
                        nc.tensor.matmul(
                            out=pz[:, kk * 128:(kk + 1) * 128],
                            lhsT=sl, rhs=zrc[:], start=True, stop=True,
                        )
                        nc.tensor.matmul(
                            out=pm[:, kk * 54:(kk + 1) * 54],
                            lhsT=hT[:, k::TPB], rhs=w2c[:], start=True, stop=True,
                        )
                    nc.vector.tensor_scalar(
                        out=S_t[:, g0:g0 + gn, :], in0=pz[:, :gn * 128],
                        scalar1=0.0, scalar2=None, op0=mybir.AluOpType.is_equal,
                    )
                    nc.vector.tensor_scalar_max(
                        w_t[:, g0:g0 + gn, 10:64], pm[:, :gn * 54], 0.0
                    )
                    nc.vector.tensor_tensor(
                        out=w_t[:, g0:g0 + gn, 0:10],
                        in0=t_bk[:, g0:g0 + gn].unsqueeze(2).to_broadcast([128, gn, 10]),
                        in1=i10[:].unsqueeze(1).to_broadcast([128, gn, 10]),
                        op=mybir.AluOpType.is_equal,
                    )
                # bf16 hi/lo split of w for the bf16 scatter matmuls
                nc.vector.tensor_copy(wh_t[:], w_t[:])
                nc.vector.scalar_tensor_tensor(
                    out=wl_t[:], in0=w_t[:], scalar=1.0, in1=wh_t[:],
                    op0=mybir.AluOpType.mult, op1=mybir.AluOpType.subtract,
                )

                # ---- segment sum: bf16 S, wh/wl accumulate into one PSUM
                pd = psdp.tile([128, 64], F32, space="PSUM", tag="pd")
                for k in range(TPB):
                    nc.tensor.matmul(
                        out=pd[:], lhsT=S_t[:, k, :], rhs=wh_t[:, k, :],
                        start=(k == 0), stop=False,
                    )
                    nc.tensor.matmul(
                        out=pd[:], lhsT=S_t[:, k, :], rhs=wl_t[:, k, :],
                        start=False, stop=(k == TPB - 1),
                    )

                # ---- r = 1/(d+eps), split to bf16 hi/lo
                dplus = wpool.tile([128, 64], F32, tag="dplus")
                nc.vector.tensor_scalar_add(dplus[:], pd[:], EPS)
                rblk = wpool.tile([128, 64], F32, tag="rblk")
                nc.vector.reciprocal(rblk[:], dplus[:])
                rhl = wpool.tile([128, 128], BF16, tag="rhl")
                nc.vector.tensor_copy(rhl[:, 0:64], rblk[:])
                nc.vector.scalar_tensor_tensor(
                    out=rhl[:, 64:128], in0=rblk[:], scalar=1.0, in1=rhl[:, 0:64],
                    op0=mybir.AluOpType.mult, op1=mybir.AluOpType.subtract,
                )

                # ---- expand reciprocals per edge + multiply + store
                # slot s = TPB*p + k lives at wt[p, k, :]; one p-major DMA
                # writes the whole bin contiguously.
                prt = psrtp.tile([128, TPB * 64], F32, space="PSUM", tag="prt")
                for k in range(TPB):
                    nc.tensor.matmul(
                        out=prt[:, k * 64:(k + 1) * 64],
                        lhsT=ST_t[:, k, :], rhs=rhl[:, 0:64],
                        start=True, stop=False,
                    )
                    nc.tensor.matmul(
                        out=prt[:, k * 64:(k + 1) * 64],
                        lhsT=ST_t[:, k, :], rhs=rhl[:, 64:128],
                        start=False, stop=True,
                    )
                wt = wpool.tile([128, TPB, 64], F32, tag="wt")
                nc.vector.tensor_tensor(
                    out=wt[:], in0=w_t[:], in1=prt[:], op=mybir.AluOpType.mult,
                )
                nc.sync.dma_start(out[e0:e0 + EPB, :], wt[:])
    nc.compile()
    return nc


# ---------------------------------------------------------------------------
# entry point
# ---------------------------------------------------------------------------

def kernel(x, edge_index, edge_attr, W1, b1, W2, b2):
    global TPB
    src = np.asarray(edge_index)[0].astype(np.int64)
    dist = np.asarray(edge_attr, np.float32)[:, 0]

    plans, tpb = plan_bins(src)
    TPB = tpb
    l1_np, w2aug_np, KH = fold_weights(W1, b1, W2, b2)
    key = (KH, TPB, NB, l1_np.tobytes(), w2aug_np.tobytes())
    nc = _NC_CACHE.get(key)
    if nc is None:
        nc = build_kernel(KH, l1_np, w2aug_np)
        _NC_CACHE[key] = nc

    in_maps, gids_all = prepare_inputs(plans, dist)
    res = run_bass_kernel_spmd(nc, in_maps, core_ids=list(range(N_CORES)))

    final = np.empty((N_EDGES, 64), np.float32)
    for k in range(N_CORES):
        o = res.results[k]["out"]
        gids = gids_all[k]
        m = gids >= 0
        final[gids[m]] = o[m]
    return final



# revision 15
# speedup vs baseline: 3.4588x; 3.4588x over previous
"""Trainium2 Bass kernel for nn_CoucheinitialeGNN (GNN edge-MLP + segment-sum normalize).

Math (reference):
    bucket = clip(int(dist), 0, 9); one_hot [E,10]
    h      = relu(dist @ W1 + b1)          [E,128]
    mlp    = relu(h @ W2 + b2)             [E,54]
    w      = concat([one_hot, mlp])        [E,64]
    d      = segment_sum(w, src, N)        [N,64]
    out    = w / d[src]   (0/0 := 0)       [E,64]

Strategy (node-major): shard nodes across 8 cores; per core sort nodes by
degree and pack 128 nodes per bin (partition = node, free axis = that
node's edges padded to the bin max degree D). The segment sum is then a
within-partition tree-reduce on the vector engine and the d[src] gather is
a free broadcast — no selection-matrix matmuls at all. The tensor engine
only computes the edge MLP: L1 as a K=9 bf16 matmul (hi/mid/lo split of
dist and W1, ~f32-exact) into an f32 hT, L2 as one fp32 [F,128]x[F,54]
matmul per 128-slot group. PRE-relu precision must track the reference's
f32 math closely: where a node's segment sum is dominated by one edge the
reference emits w/w = 1.0 for arbitrarily tiny w, so a sign flip of the
pre-relu value is a full-scale error. POST-relu everything is relative, so
w, r and the output are bf16 (|out| <= 1, tol 2e-2), halving store
traffic; the host converts to f32 and scatters rows back. Dead/always-
linear relu features are folded (and mostly promoted back) on the host.
"""

import numpy as np
import ml_dtypes

import concourse.bass as bass
import concourse.bacc as bacc
import concourse.tile as tile
import concourse.mybir as mybir
from concourse.bass_utils import run_bass_kernel_spmd

F32 = mybir.dt.float32
BF16 = mybir.dt.bfloat16

N_NODES = 100000
N_EDGES = 1600000
N_CORES = 8
THRESHOLD = 10.0

NPC = N_NODES // N_CORES                     # 12500 nodes per core
NBIN = -(-NPC // 128)                        # 98 bins of 128 nodes
EPS = 2e-38                                  # keeps 1/(d+eps) finite + normal

# L2 slot-group size: matmuls per PSUM tile / relu batch (9*54*4B < 2KB bank)
PM_JN = 9


# ---------------------------------------------------------------------------
# host-side weight folding (bitwise-identical to the proven baseline fold)
# ---------------------------------------------------------------------------

def fold_weights(W1, b1, W2, b2):
    """Split relu features into kinky (computed on device) and linear/dead
    (folded into two extra contraction rows: dist-coef and const).  Linear
    features are promoted back into the device-computed set while room
    remains so knife-edge relu channels see the exact f32 math."""
    W1 = np.asarray(W1, np.float32).reshape(-1)       # [128]
    b1 = np.asarray(b1, np.float32).reshape(-1)       # [128]
    W2 = np.asarray(W2, np.float32)                   # [128, 54]
    b2 = np.asarray(b2, np.float32).reshape(-1)       # [54]
    H, O = W2.shape

    lo = b1                                            # value at d -> 0+
    hi = THRESHOLD * W1 + b1                           # value at d = 10
    with np.errstate(divide="ignore", invalid="ignore"):
        t = np.where(W1 != 0, -b1 / W1, np.inf)
    kinky = (t > -0.5) & (t < THRESHOLD + 0.5) & (W1 != 0)
    dead = ~kinky & (np.maximum(lo, hi) <= 0)
    linear = ~kinky & ~dead                            # relu == identity on (0,10]

    room = 128 - 2 - int(kinky.sum())
    lin_idx = np.nonzero(linear)[0]
    if len(lin_idx) > room:
        impact = np.maximum(np.abs(lo), np.abs(hi))[lin_idx] * \
            np.abs(W2[lin_idx]).max(1)
        lin_idx = lin_idx[np.argsort(-impact)[:room]]
    promote = np.zeros_like(linear)
    promote[lin_idx] = True
    kinky = kinky | promote
    linear = linear & ~promote

    KH = int(kinky.sum())
    assert KH + 2 <= 128, f"kinky feature count {KH} too large"

    A = (W2[linear].astype(np.float64) * W1[linear, None].astype(np.float64)).sum(0)
    C = (W2[linear].astype(np.float64) * b1[linear, None].astype(np.float64)).sum(0) \
        + b2.astype(np.float64)

    # layer-1 lhsT [9, KH+2] bf16, paired with rhs rows
    # [dh, dh, dh, dm, dm, dl, v, v, v]:
    #   col j<KH: [wh, wm, wl, wh, wm, wh, b1h, b1m, b1l]
    #   col KH:   dist-copy [1,0,0,1,0,1,0,0,0] -> dh+dm+dl = dist
    #   col KH+1: ones-copy [0,...,0,1,0,0]     -> v (1 real / 0 pad)
    def split3(v):
        hi_ = v.astype(ml_dtypes.bfloat16)
        r = v - hi_.astype(np.float32)
        mid = r.astype(ml_dtypes.bfloat16)
        lo_ = (r - mid.astype(np.float32)).astype(ml_dtypes.bfloat16)
        return hi_, mid, lo_

    W1k = W1[kinky]
    b1k = b1[kinky]
    wh, wm, wl = split3(W1k)
    bh, bm, bl = split3(b1k)
    F = KH + 2
    l1 = np.zeros((9, F), ml_dtypes.bfloat16)
    for i, row in enumerate([wh, wm, wl, wh, wm, wh, bh, bm, bl]):
        l1[i, :KH] = row
    l1[0, KH] = 1.0
    l1[3, KH] = 1.0
    l1[5, KH] = 1.0
    l1[6, KH + 1] = 1.0

    w2aug = np.zeros((F, O), np.float32)
    w2aug[:KH] = W2[kinky]
    w2aug[KH] = A.astype(np.float32)
    w2aug[KH + 1] = C.astype(np.float32)
    # bf16 hi/lo pair of the L2 weights (used with the bf16 hi/lo pair of h:
    # x ~= hh*w2h + hh*w2l + hl*w2h, dropping hl*w2l <= 2^-18 |h||w2|)
    w2h = w2aug.astype(ml_dtypes.bfloat16)
    w2l = (w2aug - w2h.astype(np.float32)).astype(ml_dtypes.bfloat16)
    return l1, w2h, w2l, KH


# ---------------------------------------------------------------------------
# host-side edge partitioning (node-major bins)
# ---------------------------------------------------------------------------

def plan(src):
    """Sort edges by src, shard nodes across cores, sort nodes by degree and
    pack 128 per bin.  Returns per-core edge->slot data and the shared
    per-bin padded degree profile D (even, identical across cores)."""
    order = np.argsort(src, kind="stable")
    ssrc = src[order]
    bounds = np.searchsorted(ssrc, np.arange(N_CORES + 1) * NPC)
    cores = []
    Dmat = np.zeros((N_CORES, NBIN), np.int64)
    for k in range(N_CORES):
        lo, hi = bounds[k], bounds[k + 1]
        eids = order[lo:hi]
        lsrc = (ssrc[lo:hi] - k * NPC).astype(np.int64)
        deg = np.bincount(lsrc, minlength=NPC)
        nodeord = np.argsort(-deg, kind="stable")
        rank = np.empty(NPC, np.int64)
        rank[nodeord] = np.arange(NPC)
        degs = deg[nodeord]
        dpad = np.zeros(NBIN * 128, np.int64)
        dpad[:NPC] = degs
        Dmat[k] = dpad.reshape(NBIN, 128).max(1)
        starts = np.concatenate([[0], np.cumsum(deg)])
        j = np.arange(len(lsrc)) - starts[lsrc]
        cores.append({"eids": eids, "lsrc": lsrc, "rank": rank, "j": j})
    Dm = Dmat.max(0)
    D = Dm + (Dm & 1)                        # even so halving trees stay simple
    cbase = np.concatenate([[0], np.cumsum(128 * D)])
    dbase = np.concatenate([[0], np.cumsum(D)])
    return cores, D, cbase, dbase, int(cbase[-1]), int(dbase[-1])


def prepare(cores, D, cbase, dbase, EP, DSUM, dist):
    in_maps = []
    gids_all = []
    for c in cores:
        eids, lsrc, j = c["eids"], c["lsrc"], c["j"]
        r = c["rank"][lsrc]
        p = r % 128
        b = r // 128
        col = cbase[b] + j * 128 + p         # L1 rhs column of this edge
        row = cbase[b] + p * D[b] + j        # output DRAM row of this edge

        de = dist[eids]
        distv = np.zeros(EP, np.float32)
        distv[col] = de
        valid = np.zeros(EP, np.float32)
        valid[col] = 1.0
        dh = distv.astype(ml_dtypes.bfloat16)
        r1 = distv - dh.astype(np.float32)
        dm = r1.astype(ml_dtypes.bfloat16)
        dl = (r1 - dm.astype(np.float32)).astype(ml_dtypes.bfloat16)
        rhs9 = np.empty((9, EP), ml_dtypes.bfloat16)
        rhs9[0] = dh
        rhs9[1] = dh
        rhs9[2] = dh
        rhs9[3] = dm
        rhs9[4] = dm
        rhs9[5] = dl
        rhs9[6] = valid
        rhs9[7] = valid
        rhs9[8] = valid

        bucketf = np.full((128, DSUM), -1.0, ml_dtypes.bfloat16)
        bucketf[p, dbase[b] + j] = np.clip(de.astype(np.int32), 0, 9)

        gids = np.full(EP, -1, np.int64)
        gids[row] = eids
        in_maps.append({"rhs9": rhs9, "bucketf": bucketf})
        gids_all.append(gids)
    return in_maps, gids_all


# ---------------------------------------------------------------------------
# device kernel
# ---------------------------------------------------------------------------

_NC_CACHE = {}


def build_kernel(F, l1_np, w2h_np, w2l_np, D, cbase, dbase, EP, DSUM):
    nc = bacc.Bacc("TRN2", target_bir_lowering=False, debug=False, num_devices=N_CORES)

    rhs9 = nc.dram_tensor("rhs9", [9, EP], BF16, kind="ExternalInput")
    bucketf = nc.dram_tensor("bucketf", [128, DSUM], BF16, kind="ExternalInput")
    out = nc.dram_tensor("out", [EP, 64], BF16, kind="ExternalOutput")

    l1_t = nc.inline_tensor(np.ascontiguousarray(l1_np), name="l1w")
    w2h_t = nc.inline_tensor(np.ascontiguousarray(w2h_np), name="w2h")
    w2l_t = nc.inline_tensor(np.ascontiguousarray(w2l_np), name="w2l")
    iota10_np = np.broadcast_to(
        np.arange(10).astype(ml_dtypes.bfloat16), (128, 10)
    ).copy()
    i10_t = nc.inline_tensor(iota10_np, name="iota10")

    Relu = mybir.ActivationFunctionType.Relu
    ADD = mybir.AluOpType.add
    MULT = mybir.AluOpType.mult
    ISEQ = mybir.AluOpType.is_equal

    with tile.TileContext(nc) as tc, nc.allow_low_precision(
        reason="bf16 partial sums / outputs are within the 2e-2 tolerance"
    ):
        with (
            tc.tile_pool(name="const", bufs=1) as cpool,
            tc.tile_pool(name="io", bufs=3) as iopool,
            tc.tile_pool(name="work", bufs=2) as wpool,
            tc.tile_pool(name="ps1", bufs=3, space="PSUM") as ps1p,
            tc.tile_pool(name="psm", bufs=3, space="PSUM") as psmp,
        ):
            l1c = cpool.tile([9, F], BF16)
            w2hc = cpool.tile([F, 54], BF16)
            w2lc = cpool.tile([F, 54], BF16)
            i10 = cpool.tile([128, 10], BF16)
            bkall = cpool.tile([128, DSUM], BF16)
            nc.sync.dma_start(l1c[:], l1_t[:, :])
            nc.sync.dma_start(w2hc[:], w2h_t[:, :])
            nc.sync.dma_start(w2lc[:], w2l_t[:, :])
            nc.sync.dma_start(i10[:], i10_t[:, :])
            nc.sync.dma_start(bkall[:], bucketf[:, :])

            for b in range(NBIN):
                Db = int(D[b])
                if Db == 0:
                    continue
                S = 128 * Db
                e0 = int(cbase[b])
                d0 = int(dbase[b])

                t_rhs = iopool.tile([9, S], BF16, tag="rhs")
                nc.sync.dma_start(t_rhs[:], rhs9[:, e0:e0 + S])

                # ---- L1: h = relu(l1^T @ rhs9) split into a bf16 hi/lo pair
                # (hh on scalar from PSUM, hl on vector as (p1 max 0) - hh)
                hh = wpool.tile([F, S], BF16, tag="hh")
                hl = wpool.tile([F, S], BF16, tag="hl")
                c0 = 0
                while c0 < S:
                    cw = min(512, S - c0)
                    p1 = ps1p.tile([F, 512], F32, tag="p1")
                    nc.tensor.matmul(
                        out=p1[:, :cw], lhsT=l1c[:], rhs=t_rhs[:, c0:c0 + cw],
                        start=True, stop=True,
                    )
                    nc.scalar.activation(hh[:, c0:c0 + cw], p1[:, :cw], Relu)
                    nc.vector.scalar_tensor_tensor(
                        out=hl[:, c0:c0 + cw], in0=p1[:, :cw], scalar=0.0,
                        in1=hh[:, c0:c0 + cw],
                        op0=mybir.AluOpType.max, op1=mybir.AluOpType.subtract,
                    )
                    c0 += cw

                # ---- w [128, Db, 64]: one-hot(bucket) | relu(L2)
                w_t = wpool.tile([128, Db, 64], BF16, tag="w")
                nc.vector.tensor_tensor(
                    out=w_t[:, :, 0:10],
                    in0=bkall[:, d0:d0 + Db].unsqueeze(2).to_broadcast([128, Db, 10]),
                    in1=i10[:].unsqueeze(1).to_broadcast([128, Db, 10]),
                    op=ISEQ,
                )
                for j0 in range(0, Db, PM_JN):
                    jn = min(PM_JN, Db - j0)
                    pm = psmp.tile([128, PM_JN * 54], F32, tag="pm")
                    for jj in range(jn):
                        jx = j0 + jj
                        sl = slice(jx * 128, (jx + 1) * 128)
                        o = slice(jj * 54, (jj + 1) * 54)
                        nc.tensor.matmul(
                            out=pm[:, o], lhsT=hh[:, sl], rhs=w2hc[:],
                            start=True, stop=False,
                        )
                        nc.tensor.matmul(
                            out=pm[:, o], lhsT=hh[:, sl], rhs=w2lc[:],
                            start=False, stop=False,
                        )
                        nc.tensor.matmul(
                            out=pm[:, o], lhsT=hl[:, sl], rhs=w2hc[:],
                            start=False, stop=True,
                        )
                    nc.scalar.activation(
                        w_t[:, j0:j0 + jn, 10:64], pm[:, :jn * 54], Relu,
                    )

                # ---- d = sum_j w[:, j, :]  (within-partition halving tree)
                dsb = wpool.tile([128, 64], F32, tag="dsb")
                if Db == 2:
                    nc.vector.tensor_tensor(
                        out=dsb[:].unsqueeze(1),
                        in0=w_t[:, 0:1, :], in1=w_t[:, 1:2, :], op=ADD,
                    )
                else:
                    red = wpool.tile([128, Db // 2, 64], BF16, tag="red")
                    srcv = w_t
                    cur = Db
                    while cur > 2:
                        half = cur // 2
                        nc.vector.tensor_tensor(
                            out=red[:, 0:half, :],
                            in0=srcv[:, 0:half, :],
                            in1=srcv[:, half:2 * half, :],
                            op=ADD,
                        )
                        if cur & 1:
                            nc.vector.tensor_tensor(
                                out=red[:, 0:1, :],
                                in0=red[:, 0:1, :],
                                in1=srcv[:, 2 * half:cur, :],
                                op=ADD,
                            )
                        srcv = red
                        cur = half
                    if cur == 2:
                        nc.vector.tensor_tensor(
                            out=dsb[:].unsqueeze(1),
                            in0=srcv[:, 0:1, :], in1=srcv[:, 1:2, :], op=ADD,
                        )
                    else:
                        nc.vector.tensor_copy(dsb[:].unsqueeze(1), srcv[:, 0:1, :])

                # ---- r = 1/(d+eps) -> bf16
                dsb2 = wpool.tile([128, 64], F32, tag="dsb2")
                nc.vector.tensor_scalar_add(dsb2[:], dsb[:], EPS)
                rblk = wpool.tile([128, 64], F32, tag="rblk")
                nc.vector.reciprocal_approx_fast(out=rblk[:], in_=dsb2[:])
                rb16 = wpool.tile([128, 64], BF16, tag="rb16")
                nc.vector.tensor_copy(rb16[:], rblk[:])

                # ---- out = w * r (broadcast over slots), bf16 store
                out_t = wpool.tile([128, Db, 64], BF16, tag="ot")
                nc.vector.tensor_tensor(
                    out=out_t[:],
                    in0=w_t[:],
                    in1=rb16[:].unsqueeze(1).to_broadcast([128, Db, 64]),
                    op=MULT,
                )
                nc.sync.dma_start(out[e0:e0 + S, :], out_t[:])
    nc.compile()
    return nc


# ---------------------------------------------------------------------------
# entry point
# ---------------------------------------------------------------------------

def kernel(x, edge_index, edge_attr, W1, b1, W2, b2):
    src = np.asarray(edge_index)[0].astype(np.int64)
    dist = np.asarray(edge_attr, np.float32)[:, 0]

    l1_np, w2h_np, w2l_np, KH = fold_weights(W1, b1, W2, b2)
    F = KH + 2
    cores, D, cbase, dbase, EP, DSUM = plan(src)

    key = (F, D.tobytes(), l1_np.tobytes(), w2h_np.tobytes())
    nc = _NC_CACHE.get(key)
    if nc is None:
        nc = build_kernel(F, l1_np, w2h_np, w2l_np, D, cbase, dbase, EP, DSUM)
        _NC_CACHE[key] = nc

    in_maps, gids_all = prepare(cores, D, cbase, dbase, EP, DSUM, dist)
    res = run_bass_kernel_spmd(nc, in_maps, core_ids=list(range(N_CORES)))

    final = np.empty((N_EDGES, 64), np.float32)
    for k in range(N_CORES):
        o = np.asarray(res.results[k]["out"]).astype(np.float32)
        gids = gids_all[k]
        m = gids >= 0
        final[gids[m]] = o[m]
    return final


# revision 17
# speedup vs baseline: 3.4703x; 1.0033x over previous
"""Trainium2 Bass kernel for nn_CoucheinitialeGNN (GNN edge-MLP + segment-sum normalize).

Math (reference):
    bucket = clip(int(dist), 0, 9); one_hot [E,10]
    h      = relu(dist @ W1 + b1)          [E,128]
    mlp    = relu(h @ W2 + b2)             [E,54]
    w      = concat([one_hot, mlp])        [E,64]
    d      = segment_sum(w, src, N)        [N,64]
    out    = w / d[src]   (0/0 := 0)       [E,64]

Strategy (node-major): shard nodes across 8 cores; per core sort nodes by
degree and pack 128 nodes per bin (partition = node, free axis = that
node's edges padded to the bin max degree D). The segment sum is then a
within-partition tree-reduce on the vector engine and the d[src] gather is
a free broadcast — no selection-matrix matmuls at all. The tensor engine
only computes the edge MLP: L1 as a K=9 bf16 matmul (hi/mid/lo split of
dist and W1, ~f32-exact) into an f32 hT, L2 as one fp32 [F,128]x[F,54]
matmul per 128-slot group. PRE-relu precision must track the reference's
f32 math closely: where a node's segment sum is dominated by one edge the
reference emits w/w = 1.0 for arbitrarily tiny w, so a sign flip of the
pre-relu value is a full-scale error. POST-relu everything is relative, so
w, r and the output are bf16 (|out| <= 1, tol 2e-2), halving store
traffic; the host converts to f32 and scatters rows back. Dead/always-
linear relu features are folded (and mostly promoted back) on the host.
"""

import numpy as np
import ml_dtypes

import concourse.bass as bass
import concourse.bacc as bacc
import concourse.tile as tile
import concourse.mybir as mybir
from concourse.bass_utils import run_bass_kernel_spmd

F32 = mybir.dt.float32
BF16 = mybir.dt.bfloat16

N_NODES = 100000
N_EDGES = 1600000
N_CORES = 8
THRESHOLD = 10.0

NPC = N_NODES // N_CORES                     # 12500 nodes per core
NBIN = -(-NPC // 128)                        # 98 bins of 128 nodes
EPS = 2e-38                                  # keeps 1/(d+eps) finite + normal

# L2 slot-group size: matmuls per PSUM tile / relu batch (9*54*4B < 2KB bank)
PM_JN = 9


# ---------------------------------------------------------------------------
# host-side weight folding (bitwise-identical to the proven baseline fold)
# ---------------------------------------------------------------------------

def fold_weights(W1, b1, W2, b2):
    """Split relu features into kinky (computed on device) and linear/dead
    (folded into two extra contraction rows: dist-coef and const).  Linear
    features are promoted back into the device-computed set while room
    remains so knife-edge relu channels see the exact f32 math."""
    W1 = np.asarray(W1, np.float32).reshape(-1)       # [128]
    b1 = np.asarray(b1, np.float32).reshape(-1)       # [128]
    W2 = np.asarray(W2, np.float32)                   # [128, 54]
    b2 = np.asarray(b2, np.float32).reshape(-1)       # [54]
    H, O = W2.shape

    lo = b1                                            # value at d -> 0+
    hi = THRESHOLD * W1 + b1                           # value at d = 10
    with np.errstate(divide="ignore", invalid="ignore"):
        t = np.where(W1 != 0, -b1 / W1, np.inf)
    kinky = (t > -0.5) & (t < THRESHOLD + 0.5) & (W1 != 0)
    dead = ~kinky & (np.maximum(lo, hi) <= 0)
    linear = ~kinky & ~dead                            # relu == identity on (0,10]

    room = 128 - 2 - int(kinky.sum())
    lin_idx = np.nonzero(linear)[0]
    if len(lin_idx) > room:
        impact = np.maximum(np.abs(lo), np.abs(hi))[lin_idx] * \
            np.abs(W2[lin_idx]).max(1)
        lin_idx = lin_idx[np.argsort(-impact)[:room]]
    promote = np.zeros_like(linear)
    promote[lin_idx] = True
    kinky = kinky | promote
    linear = linear & ~promote

    KH = int(kinky.sum())
    assert KH + 2 <= 128, f"kinky feature count {KH} too large"

    A = (W2[linear].astype(np.float64) * W1[linear, None].astype(np.float64)).sum(0)
    C = (W2[linear].astype(np.float64) * b1[linear, None].astype(np.float64)).sum(0) \
        + b2.astype(np.float64)

    # layer-1 lhsT [9, KH+2] bf16, paired with rhs rows
    # [dh, dh, dh, dm, dm, dl, v, v, v]:
    #   col j<KH: [wh, wm, wl, wh, wm, wh, b1h, b1m, b1l]
    #   col KH:   dist-copy [1,0,0,1,0,1,0,0,0] -> dh+dm+dl = dist
    #   col KH+1: ones-copy [0,...,0,1,0,0]     -> v (1 real / 0 pad)
    def split3(v):
        hi_ = v.astype(ml_dtypes.bfloat16)
        r = v - hi_.astype(np.float32)
        mid = r.astype(ml_dtypes.bfloat16)
        lo_ = (r - mid.astype(np.float32)).astype(ml_dtypes.bfloat16)
        return hi_, mid, lo_

    W1k = W1[kinky]
    b1k = b1[kinky]
    wh, wm, wl = split3(W1k)
    bh, bm, bl = split3(b1k)
    F = KH + 2
    l1 = np.zeros((9, F), ml_dtypes.bfloat16)
    for i, row in enumerate([wh, wm, wl, wh, wm, wh, bh, bm, bl]):
        l1[i, :KH] = row
    l1[0, KH] = 1.0
    l1[3, KH] = 1.0
    l1[5, KH] = 1.0
    l1[6, KH + 1] = 1.0

    w2aug = np.zeros((F, O), np.float32)
    w2aug[:KH] = W2[kinky]
    w2aug[KH] = A.astype(np.float32)
    w2aug[KH + 1] = C.astype(np.float32)
    # bf16 hi/lo pair of the L2 weights (used with the bf16 hi/lo pair of h:
    # x ~= hh*w2h + hh*w2l + hl*w2h, dropping hl*w2l <= 2^-18 |h||w2|)
    w2h = w2aug.astype(ml_dtypes.bfloat16)
    w2l = (w2aug - w2h.astype(np.float32)).astype(ml_dtypes.bfloat16)
    return l1, w2h, w2l, KH


# ---------------------------------------------------------------------------
# host-side edge partitioning (node-major bins)
# ---------------------------------------------------------------------------

def plan(src):
    """Sort edges by src, shard nodes across cores, sort nodes by degree and
    pack 128 per bin.  Returns per-core edge->slot data and the shared
    per-bin padded degree profile D (even, identical across cores)."""
    order = np.argsort(src, kind="stable")
    ssrc = src[order]
    bounds = np.searchsorted(ssrc, np.arange(N_CORES + 1) * NPC)
    cores = []
    Dmat = np.zeros((N_CORES, NBIN), np.int64)
    for k in range(N_CORES):
        lo, hi = bounds[k], bounds[k + 1]
        eids = order[lo:hi]
        lsrc = (ssrc[lo:hi] - k * NPC).astype(np.int64)
        deg = np.bincount(lsrc, minlength=NPC)
        nodeord = np.argsort(-deg, kind="stable")
        rank = np.empty(NPC, np.int64)
        rank[nodeord] = np.arange(NPC)
        degs = deg[nodeord]
        dpad = np.zeros(NBIN * 128, np.int64)
        dpad[:NPC] = degs
        Dmat[k] = dpad.reshape(NBIN, 128).max(1)
        starts = np.concatenate([[0], np.cumsum(deg)])
        j = np.arange(len(lsrc)) - starts[lsrc]
        cores.append({"eids": eids, "lsrc": lsrc, "rank": rank, "j": j})
    Dm = Dmat.max(0)
    D = Dm + (Dm & 1)                        # even so halving trees stay simple
    cbase = np.concatenate([[0], np.cumsum(128 * D)])
    dbase = np.concatenate([[0], np.cumsum(D)])
    return cores, D, cbase, dbase, int(cbase[-1]), int(dbase[-1])


def prepare(cores, D, cbase, dbase, EP, DSUM, dist):
    in_maps = []
    gids_all = []
    for c in cores:
        eids, lsrc, j = c["eids"], c["lsrc"], c["j"]
        r = c["rank"][lsrc]
        p = r % 128
        b = r // 128
        col = cbase[b] + j * 128 + p         # L1 rhs column of this edge
        row = cbase[b] + p * D[b] + j        # output DRAM row of this edge

        de = dist[eids]
        distv = np.zeros(EP, np.float32)
        distv[col] = de
        valid = np.zeros(EP, np.float32)
        valid[col] = 1.0
        dh = distv.astype(ml_dtypes.bfloat16)
        r1 = distv - dh.astype(np.float32)
        dm = r1.astype(ml_dtypes.bfloat16)
        dl = (r1 - dm.astype(np.float32)).astype(ml_dtypes.bfloat16)
        rhs9 = np.empty((9, EP), ml_dtypes.bfloat16)
        rhs9[0] = dh
        rhs9[1] = dh
        rhs9[2] = dh
        rhs9[3] = dm
        rhs9[4] = dm
        rhs9[5] = dl
        rhs9[6] = valid
        rhs9[7] = valid
        rhs9[8] = valid

        bucketf = np.full((128, DSUM), -1.0, ml_dtypes.bfloat16)
        bucketf[p, dbase[b] + j] = np.clip(de.astype(np.int32), 0, 9)

        gids = np.full(EP, -1, np.int64)
        gids[row] = eids
        in_maps.append({"rhs9": rhs9, "bucketf": bucketf})
        gids_all.append(gids)
    return in_maps, gids_all


# ---------------------------------------------------------------------------
# device kernel
# ---------------------------------------------------------------------------

_NC_CACHE = {}


def build_kernel(F, l1_np, w2h_np, w2l_np, D, cbase, dbase, EP, DSUM):
    nc = bacc.Bacc("TRN2", target_bir_lowering=False, debug=False, num_devices=N_CORES)

    rhs9 = nc.dram_tensor("rhs9", [9, EP], BF16, kind="ExternalInput")
    bucketf = nc.dram_tensor("bucketf", [128, DSUM], BF16, kind="ExternalInput")
    out = nc.dram_tensor("out", [EP, 64], BF16, kind="ExternalOutput")

    l1_t = nc.inline_tensor(np.ascontiguousarray(l1_np), name="l1w")
    w2h_t = nc.inline_tensor(np.ascontiguousarray(w2h_np), name="w2h")
    w2l_t = nc.inline_tensor(np.ascontiguousarray(w2l_np), name="w2l")
    iota10_np = np.broadcast_to(
        np.arange(10).astype(ml_dtypes.bfloat16), (128, 10)
    ).copy()
    i10_t = nc.inline_tensor(iota10_np, name="iota10")

    Relu = mybir.ActivationFunctionType.Relu
    ADD = mybir.AluOpType.add
    MULT = mybir.AluOpType.mult
    ISEQ = mybir.AluOpType.is_equal

    with tile.TileContext(nc) as tc, nc.allow_low_precision(
        reason="bf16 partial sums / outputs are within the 2e-2 tolerance"
    ):
        with (
            tc.tile_pool(name="const", bufs=1) as cpool,
            tc.tile_pool(name="io", bufs=3) as iopool,
            tc.tile_pool(name="work", bufs=3) as wpool,
            tc.tile_pool(name="ps1", bufs=3, space="PSUM") as ps1p,
            tc.tile_pool(name="psm", bufs=3, space="PSUM") as psmp,
        ):
            l1c = cpool.tile([9, F], BF16)
            w2hc = cpool.tile([F, 54], BF16)
            w2lc = cpool.tile([F, 54], BF16)
            i10 = cpool.tile([128, 10], BF16)
            bkall = cpool.tile([128, DSUM], BF16)
            nc.sync.dma_start(l1c[:], l1_t[:, :])
            nc.sync.dma_start(w2hc[:], w2h_t[:, :])
            nc.sync.dma_start(w2lc[:], w2l_t[:, :])
            nc.sync.dma_start(i10[:], i10_t[:, :])
            nc.sync.dma_start(bkall[:], bucketf[:, :])

            for b in range(NBIN):
                Db = int(D[b])
                if Db == 0:
                    continue
                S = 128 * Db
                e0 = int(cbase[b])
                d0 = int(dbase[b])

                t_rhs = iopool.tile([9, S], BF16, tag="rhs")
                nc.sync.dma_start(t_rhs[:], rhs9[:, e0:e0 + S])

                # ---- L1: h = relu(l1^T @ rhs9) split into a bf16 hi/lo pair
                # (hh on scalar from PSUM, hl on vector as (p1 max 0) - hh)
                hh = wpool.tile([F, S], BF16, tag="hh")
                hl = wpool.tile([F, S], BF16, tag="hl")
                c0 = 0
                while c0 < S:
                    cw = min(512, S - c0)
                    p1 = ps1p.tile([F, 512], F32, tag="p1")
                    nc.tensor.matmul(
                        out=p1[:, :cw], lhsT=l1c[:], rhs=t_rhs[:, c0:c0 + cw],
                        start=True, stop=True,
                    )
                    nc.scalar.activation(hh[:, c0:c0 + cw], p1[:, :cw], Relu)
                    nc.vector.scalar_tensor_tensor(
                        out=hl[:, c0:c0 + cw], in0=p1[:, :cw], scalar=0.0,
                        in1=hh[:, c0:c0 + cw],
                        op0=mybir.AluOpType.max, op1=mybir.AluOpType.subtract,
                    )
                    c0 += cw

                # ---- w [128, Db, 64]: one-hot(bucket) | relu(L2)
                w_t = wpool.tile([128, Db, 64], BF16, tag="w")
                nc.vector.tensor_tensor(
                    out=w_t[:, :, 0:10],
                    in0=bkall[:, d0:d0 + Db].unsqueeze(2).to_broadcast([128, Db, 10]),
                    in1=i10[:].unsqueeze(1).to_broadcast([128, Db, 10]),
                    op=ISEQ,
                )
                for j0 in range(0, Db, PM_JN):
                    jn = min(PM_JN, Db - j0)
                    pm = psmp.tile([128, PM_JN * 54], F32, tag="pm")
                    for jj in range(jn):
                        jx = j0 + jj
                        sl = slice(jx * 128, (jx + 1) * 128)
                        o = slice(jj * 54, (jj + 1) * 54)
                        nc.tensor.matmul(
                            out=pm[:, o], lhsT=hh[:, sl], rhs=w2hc[:],
                            start=True, stop=False,
                        )
                        nc.tensor.matmul(
                            out=pm[:, o], lhsT=hh[:, sl], rhs=w2lc[:],
                            start=False, stop=False,
                        )
                        nc.tensor.matmul(
                            out=pm[:, o], lhsT=hl[:, sl], rhs=w2hc[:],
                            start=False, stop=True,
                        )
                    nc.scalar.activation(
                        w_t[:, j0:j0 + jn, 10:64], pm[:, :jn * 54], Relu,
                    )

                # ---- d+eps = eps + sum_j w[:, j, :] (within-partition tree;
                # eps fused into the final add via scalar_tensor_tensor)
                dsb = wpool.tile([128, 64], F32, tag="dsb")

                def final_add(a_ap, b_ap):
                    nc.vector.scalar_tensor_tensor(
                        out=dsb[:].unsqueeze(1), in0=a_ap, scalar=EPS,
                        in1=b_ap, op0=ADD, op1=ADD,
                    )

                if Db == 2:
                    final_add(w_t[:, 0:1, :], w_t[:, 1:2, :])
                else:
                    red = wpool.tile([128, Db // 2, 64], BF16, tag="red")
                    srcv = w_t
                    cur = Db
                    while cur > 2:
                        half = cur // 2
                        nc.vector.tensor_tensor(
                            out=red[:, 0:half, :],
                            in0=srcv[:, 0:half, :],
                            in1=srcv[:, half:2 * half, :],
                            op=ADD,
                        )
                        if cur & 1:
                            nc.vector.tensor_tensor(
                                out=red[:, 0:1, :],
                                in0=red[:, 0:1, :],
                                in1=srcv[:, 2 * half:cur, :],
                                op=ADD,
                            )
                        srcv = red
                        cur = half
                    if cur == 2:
                        final_add(srcv[:, 0:1, :], srcv[:, 1:2, :])
                    else:
                        nc.vector.tensor_scalar_add(
                            dsb[:].unsqueeze(1), srcv[:, 0:1, :], EPS,
                        )

                # ---- r = 1/(d+eps) f32 (mult below is 1x either way)
                rblk = wpool.tile([128, 64], F32, tag="rblk")
                nc.vector.reciprocal_approx_fast(out=rblk[:], in_=dsb[:])

                # ---- out = w * r (broadcast over slots), bf16 store
                out_t = wpool.tile([128, Db, 64], BF16, tag="ot")
                nc.vector.tensor_tensor(
                    out=out_t[:],
                    in0=w_t[:],
                    in1=rblk[:].unsqueeze(1).to_broadcast([128, Db, 64]),
                    op=MULT,
                )
                nc.sync.dma_start(out[e0:e0 + S, :], out_t[:])
    nc.compile()
    return nc


# ---------------------------------------------------------------------------
# entry point
# ---------------------------------------------------------------------------

def kernel(x, edge_index, edge_attr, W1, b1, W2, b2):
    src = np.asarray(edge_index)[0].astype(np.int64)
    dist = np.asarray(edge_attr, np.float32)[:, 0]

    l1_np, w2h_np, w2l_np, KH = fold_weights(W1, b1, W2, b2)
    F = KH + 2
    cores, D, cbase, dbase, EP, DSUM = plan(src)

    key = (F, D.tobytes(), l1_np.tobytes(), w2h_np.tobytes())
    nc = _NC_CACHE.get(key)
    if nc is None:
        nc = build_kernel(F, l1_np, w2h_np, w2l_np, D, cbase, dbase, EP, DSUM)
        _NC_CACHE[key] = nc

    in_maps, gids_all = prepare(cores, D, cbase, dbase, EP, DSUM, dist)
    res = run_bass_kernel_spmd(nc, in_maps, core_ids=list(range(N_CORES)))

    final = np.empty((N_EDGES, 64), np.float32)
    for k in range(N_CORES):
        o = np.asarray(res.results[k]["out"]).astype(np.float32)
        gids = gids_all[k]
        m = gids >= 0
        final[gids[m]] = o[m]
    return final
